# revision 1
# baseline (speedup 1.0000x reference)
"""Trainium2 Bass kernel for MLP-with-SOM-cosine-similarity (retrieval_knn).

Reference computation per (b, k) pair:
  ctx, ent: [L=128, D=128] slices of context[b, k, 0/1]
  sim[l, m] = cos(ctx[l], ent[m]); idx[l] = argmax_m sim[l, m]
  x = [ctx_n | ent_n[idx]] -> 6x tanh(Linear(256,256)) -> dot W_out -> sum over l
Output: [B=64, K=64] f32.

Strategy: data-parallel over batch dim (8 cores x 8 batches = 512 pairs/core).
Per pair on-device: row norms via Newton rsqrt (no ACT table switch), normalize
on gpsimd, PE transposes to feature-major layout, fp32 PE matmul for the
similarity (bf16 would flip argmax picks), argmax as reduce_max + is_equal
one-hot, gather as one-hot matmul, MLP in bf16 with fp32 PSUM accumulation,
tanh on ACT batched over 8 pairs (bias fused), final dot via matmul with W_out
+ segmented free-dim reduce.
"""

from contextlib import ExitStack

import numpy as np
import ml_dtypes

import concourse.bass as bass
import concourse.bacc as bacc
import concourse.tile as tile
from concourse import mybir
from concourse.alu_op_type import AluOpType
from concourse.bass_utils import run_bass_kernel_spmd
from concourse.masks import make_identity

BF16 = mybir.dt.bfloat16
F32 = mybir.dt.float32
AF = mybir.ActivationFunctionType

B, K, L, D = 64, 64, 128, 128
N_CORES = 8
PAIRS = (B // N_CORES) * K          # 512 pairs per core
N_HIDDEN = 6
SUB = 16                            # pairs per DMA subgroup
GRP = 4                             # pairs per PSUM group
UNROLL = 128                        # pairs per outer block

_cache = {}


def _build_bass():
    nc = bacc.Bacc("TRN2")

    ctx_dram = nc.dram_tensor("ctxpairs", [PAIRS, 2, L, D], F32, kind="ExternalInput")
    wt_dram = nc.dram_tensor("wt", [128, N_HIDDEN * 2 * 2 * 128], BF16, kind="ExternalInput")
    wout_dram = nc.dram_tensor("wout", [128, 2], BF16, kind="ExternalInput")
    bias_dram = nc.dram_tensor("bias", [128, N_HIDDEN * 2], F32, kind="ExternalInput")
    bout_dram = nc.dram_tensor("bout", [1, 1], F32, kind="ExternalInput")
    out_dram = nc.dram_tensor("out", [1, PAIRS], F32, kind="ExternalOutput")

    with ExitStack() as ctx:
        tc = ctx.enter_context(tile.TileContext(nc))
        const = ctx.enter_context(tc.tile_pool(name="const", bufs=1))
        raw_pool = ctx.enter_context(tc.tile_pool(name="raw", bufs=3))
        nrm_pool = ctx.enter_context(tc.tile_pool(name="nrm", bufs=2))
        tiny_pool = ctx.enter_context(tc.tile_pool(name="tiny", bufs=4))
        trash_pool = ctx.enter_context(tc.tile_pool(name="trash", bufs=2))
        norm_sb = ctx.enter_context(tc.tile_pool(name="normsb", bufs=2))
        pre_sb = ctx.enter_context(tc.tile_pool(name="presb", bufs=3))
        x_pool = ctx.enter_context(tc.tile_pool(name="xsb", bufs=3))
        y_pool = ctx.enter_context(tc.tile_pool(name="ysb", bufs=3))
        # PSUM: 8 banks total — tp(2) + scr(2) + mlp(4)
        ps_tp = ctx.enter_context(tc.tile_pool(name="pstp", bufs=1, space="PSUM"))
        ps_scr = ctx.enter_context(tc.tile_pool(name="psscr", bufs=2, space="PSUM"))
        ps_mlp = ctx.enter_context(tc.tile_pool(name="psmlp", bufs=2, space="PSUM"))

        wt_sb = const.tile([128, N_HIDDEN, 2, 2, 128], BF16)
        nc.sync.dma_start(out=wt_sb, in_=wt_dram.rearrange("a (i kc mc b) -> a i kc mc b", i=N_HIDDEN, kc=2, mc=2))
        wout_sb = const.tile([128, 2], BF16)
        nc.sync.dma_start(out=wout_sb, in_=wout_dram[:, :])
        bias_sb = const.tile([128, N_HIDDEN * 2], F32)
        nc.sync.dma_start(out=bias_sb, in_=bias_dram[:, :])
        bout_sb = const.tile([1, 1], F32)
        nc.sync.dma_start(out=bout_sb, in_=bout_dram[:, :])
        ident = const.tile([128, 128], F32)
        make_identity(nc, ident)
        identb = const.tile([128, 128], BF16)
        make_identity(nc, identb)
        bout128 = const.tile([1, 1], F32)
        nc.vector.tensor_scalar(out=bout128, in0=bout_sb, scalar1=float(L), scalar2=0.0,
                                op0=AluOpType.mult, op1=AluOpType.add)

        n_sub = UNROLL // SUB

        for g0 in range(0, PAIRS, UNROLL):
            res = nrm_pool.tile([1, UNROLL], F32, tag="res")
            for s in range(n_sub):
                raw = raw_pool.tile([128, SUB, 2, 128], F32, tag="raw")
                nc.sync.dma_start(
                    out=raw,
                    in_=ctx_dram[g0 + s * SUB : g0 + s * SUB + SUB].rearrange("p c l d -> l p c d"),
                )

                # --- norms^2 per (pair, ctx/ent) -> [128, 2*SUB]
                sq = trash_pool.tile([128, SUB, 2, 128], F32, tag="sq")
                nc.gpsimd.tensor_mul(sq, raw, raw)
                nrm2 = nrm_pool.tile([128, 2 * SUB], F32, tag="nrm2")
                nc.vector.tensor_reduce(
                    nrm2.rearrange("a (p c) -> a p c", p=SUB),
                    sq, axis=mybir.AxisListType.X, op=AluOpType.add,
                )
                nc.vector.tensor_scalar(out=nrm2, in0=nrm2, scalar1=1.0 / 128.0,
                                        scalar2=0.0, op0=AluOpType.mult, op1=AluOpType.add)

                # --- rinv = 1/sqrt(nrm2*128) via Newton on x' = nrm2 ~ 1
                yv = tiny_pool.tile([128, 2 * SUB], F32, tag="newty")
                tv = tiny_pool.tile([128, 2 * SUB], F32, tag="newtt")
                nc.vector.tensor_scalar(out=yv, in0=nrm2, scalar1=-0.5, scalar2=1.5,
                                        op0=AluOpType.mult, op1=AluOpType.add)
                for _ in range(3):
                    nc.vector.tensor_mul(tv, yv, yv)
                    nc.vector.tensor_mul(tv, tv, nrm2)
                    nc.vector.tensor_scalar(out=tv, in0=tv, scalar1=-0.5, scalar2=1.5,
                                            op0=AluOpType.mult, op1=AluOpType.add)
                    nc.vector.tensor_mul(yv, yv, tv)
                rinv = tiny_pool.tile([128, 2 * SUB], F32, tag="rinv")
                nc.vector.tensor_scalar(out=rinv, in0=yv, scalar1=float(1.0 / np.sqrt(128.0)),
                                        scalar2=0.0, op0=AluOpType.mult, op1=AluOpType.add)

                # --- normalized rows (f32 for transposes, bf16 ent for gather lhsT)
                ctxn = norm_sb.tile([128, SUB, 128], F32, tag="ctxn")
                entn = norm_sb.tile([128, SUB, 128], F32, tag="entn")
                entnb = norm_sb.tile([128, SUB, 128], BF16, tag="entnb")
                for p in range(SUB):
                    nc.gpsimd.tensor_scalar_mul(ctxn[:, p, :], raw[:, p, 0, :], rinv[:, 2 * p : 2 * p + 1])
                    nc.gpsimd.tensor_scalar_mul(entn[:, p, :], raw[:, p, 1, :], rinv[:, 2 * p + 1 : 2 * p + 2])
                for p in range(SUB):
                    nc.vector.tensor_copy(entnb[:, p, :], entn[:, p, :])

                x_tiles = []
                for q in range(SUB // GRP):
                    pbase = q * GRP
                    # --- transposes: ctx_nT, ent_nT (fp32, PE)
                    tpc = ps_tp.tile([128, GRP, 128], F32, tag="tpc")
                    tpe = ps_tp.tile([128, GRP, 128], F32, tag="tpe")
                    for j in range(GRP):
                        nc.tensor.transpose(tpc[:, j, :], ctxn[:, pbase + j, :], ident)
                    for j in range(GRP):
                        nc.tensor.transpose(tpe[:, j, :], entn[:, pbase + j, :], ident)
                    ctxnT = pre_sb.tile([128, GRP, 128], F32, tag="ctxnT")
                    entnT = pre_sb.tile([128, GRP, 128], F32, tag="entnT")
                    nc.vector.tensor_copy(ctxnT, tpc)
                    nc.vector.tensor_copy(entnT, tpe)

                    x_sb = x_pool.tile([128, 2, GRP, 128], BF16, tag="x")
                    nc.vector.tensor_copy(x_sb[:, 0], ctxnT)  # chunk0 bf16

                    # --- similarity (fp32) + argmax one-hot
                    gps = ps_scr.tile([128, GRP, 128], F32, tag="scr")
                    for j in range(GRP):
                        nc.tensor.matmul(gps[:, j, :], lhsT=ctxnT[:, j, :], rhs=entnT[:, j, :])
                    mx = tiny_pool.tile([128, GRP], F32, tag="mx")
                    nc.vector.tensor_reduce(mx, gps, axis=mybir.AxisListType.X, op=AluOpType.max)
                    oh = pre_sb.tile([128, GRP, 128], BF16, tag="oh")
                    nc.vector.tensor_tensor(
                        out=oh, in0=gps,
                        in1=mx.unsqueeze(2).broadcast_to([128, GRP, 128]),
                        op=AluOpType.is_equal,
                    )
                    # --- transpose one-hot (bf16) and gather = ent_n^T @ onehot^T
                    ohT_ps = ps_scr.tile([128, GRP, 128], BF16, tag="scr")
                    for j in range(GRP):
                        nc.tensor.transpose(ohT_ps[:, j, :], oh[:, j, :], identb)
                    ohT = pre_sb.tile([128, GRP, 128], BF16, tag="ohTsb")
                    nc.vector.tensor_copy(ohT, ohT_ps)
                    ch1 = ps_scr.tile([128, GRP, 128], F32, tag="scr")
                    for j in range(GRP):
                        nc.tensor.matmul(ch1[:, j, :], lhsT=entnb[:, pbase + j, :], rhs=ohT[:, j, :])
                    nc.vector.tensor_copy(x_sb[:, 1], ch1)  # chunk1 bf16
                    x_tiles.append(x_sb)

                # --- MLP over 8-pair super-groups: tanh batched [128, 1024]
                for qq in range(SUB // (2 * GRP)):
                    xin = [
                        [x_tiles[2 * qq + g][:, kc].rearrange("a g d -> a (g d)") for kc in range(2)]
                        for g in range(2)
                    ]
                    for i in range(N_HIDDEN):
                        ya = y_pool.tile([128, 2, 2, GRP * 128], BF16, tag="y")
                        for mc in range(2):
                            mm = ps_mlp.tile([128, 2, GRP * 128], F32, tag="mm")
                            for g in range(2):
                                nc.tensor.matmul(mm[:, g, :], lhsT=wt_sb[:, i, 0, mc, :],
                                                 rhs=xin[g][0], start=True, stop=False)
                                nc.tensor.matmul(mm[:, g, :], lhsT=wt_sb[:, i, 1, mc, :],
                                                 rhs=xin[g][1], start=False, stop=True)
                            nc.scalar.activation(
                                out=ya[:, mc].rearrange("a g d -> a (g d)"),
                                in_=mm.rearrange("a g d -> a (g d)"),
                                func=AF.Tanh,
                                bias=bias_sb[:, 2 * i + mc : 2 * i + mc + 1],
                            )
                        xin = [[ya[:, kc, g] for kc in range(2)] for g in range(2)]

                    # --- out = W_out . x7 summed over rows per pair
                    for g in range(2):
                        wo = ps_scr.tile([1, GRP * 128], F32, tag="scr")
                        nc.tensor.matmul(wo, lhsT=wout_sb[:, 0:1], rhs=xin[g][0],
                                         start=True, stop=False)
                        nc.tensor.matmul(wo, lhsT=wout_sb[:, 1:2], rhs=xin[g][1],
                                         start=False, stop=True)
                        col = s * SUB + (2 * qq + g) * GRP
                        nc.vector.tensor_reduce(
                            res[0:1, col : col + GRP],
                            wo.rearrange("a (g d) -> a g d", g=GRP),
                            axis=mybir.AxisListType.X, op=AluOpType.add,
                        )

            # res += L * b_out  (sum over L rows of constant bias)
            nc.vector.tensor_scalar(out=res, in0=res, scalar1=bout128[0:1, 0:1], scalar2=0.0,
                                    op0=AluOpType.add, op1=AluOpType.add)
            nc.sync.dma_start(out=out_dram[0:1, g0 : g0 + UNROLL], in_=res)

    nc.compile()
    return nc


def _prep_weights(Ws, bs, W_out, b_out):
    Ws = np.asarray(Ws, dtype=np.float32)
    bs = np.asarray(bs, dtype=np.float32)
    W_out = np.asarray(W_out, dtype=np.float32)
    b_out = np.asarray(b_out, dtype=np.float32)
    # wt[a, i, kc, mc, b] = Ws[i, mc*128+b, kc*128+a]
    wt = np.transpose(
        Ws.reshape(N_HIDDEN, 2, 128, 2, 128),  # [i, mc, b, kc, a]
        (4, 0, 3, 1, 2),
    ).reshape(128, N_HIDDEN * 2 * 2 * 128)
    wt = np.ascontiguousarray(wt.astype(ml_dtypes.bfloat16))
    wout = np.ascontiguousarray(W_out.reshape(2, 128).T.astype(ml_dtypes.bfloat16))
    bias = np.ascontiguousarray(
        np.transpose(bs.reshape(N_HIDDEN, 2, 128), (2, 0, 1)).reshape(128, N_HIDDEN * 2)
    ).astype(np.float32)
    bout = b_out.reshape(1, 1).astype(np.float32)
    return wt, wout, bias, bout


def make_in_maps(context, Ws, bs, W_out, b_out):
    context = np.ascontiguousarray(np.asarray(context, dtype=np.float32))
    wt, wout, bias, bout = _prep_weights(Ws, bs, W_out, b_out)
    shards = context.reshape(N_CORES, PAIRS, 2, L, D)
    return [
        {"ctxpairs": np.ascontiguousarray(shards[i]), "wt": wt, "wout": wout,
         "bias": bias, "bout": bout}
        for i in range(N_CORES)
    ]


def kernel(context, Ws, bs, W_out, b_out):
    in_maps = make_in_maps(context, Ws, bs, W_out, b_out)
    if "nc" not in _cache:
        _cache["nc"] = _build_bass()
    nc = _cache["nc"]
    r = run_bass_kernel_spmd(nc, in_maps, core_ids=list(range(N_CORES)))
    out = np.concatenate([r.results[i]["out"].reshape(B // N_CORES, K) for i in range(N_CORES)], axis=0)
    return out.astype(np.float32)


if __name__ == "__main__":
    import reference
    inputs = reference.setup_inputs()
    inputs = {k: np.asarray(v) for k, v in inputs.items()}
    expected = np.asarray(reference.reference(**inputs))
    actual = kernel(**inputs)
    err = np.linalg.norm(actual - expected) / np.linalg.norm(expected)
    print("Relative error:", err)



# revision 2
# speedup vs baseline: 1.8000x; 1.8000x over previous
"""Trainium2 Bass kernel for MLP-with-SOM-cosine-similarity (retrieval_knn).

Reference computation per (b, k) pair:
  ctx, ent: [L=128, D=128] slices of context[b, k, 0/1]
  sim[l, m] = cos(ctx[l], ent[m]); idx[l] = argmax_m sim[l, m]
  x = [ctx_n | ent_n[idx]] -> 6x tanh(Linear(256,256)) -> dot W_out -> sum over l
Output: [B=64, K=64] f32.

Strategy: data-parallel over batch dim (8 cores x 8 batches = 512 pairs/core).
Per pair on-device: row norms via Newton rsqrt (no ACT table switch), normalize
as ONE DVE broadcast-multiply per 16-pair block (gpsimd tensor_scalar is ~16x
slower than its cost model - measured), PE transposes to feature-major layout,
fp32 PE matmul for the similarity (fp16/bf16 flip argmax picks -> rel err
>1.7e-2), argmax as reduce_max + is_equal one-hot, gather as one-hot matmul,
MLP in bf16 with fp32 PSUM accumulation (fp8 fails: 8.9e-2 rel err), tanh on
ACT batched over 8 pairs (bias fused), x0 cast on ACT (Copy shares the tanh
table - no table switch), final dot via matmul with W_out + segmented free-dim
reduce.
"""

from contextlib import ExitStack

import numpy as np
import ml_dtypes

import concourse.bass as bass
import concourse.bacc as bacc
import concourse.tile as tile
from concourse import mybir
from concourse.alu_op_type import AluOpType
from concourse.bass_utils import run_bass_kernel_spmd
from concourse.masks import make_identity

BF16 = mybir.dt.bfloat16
F32 = mybir.dt.float32
AF = mybir.ActivationFunctionType

B, K, L, D = 64, 64, 128, 128
N_CORES = 8
PAIRS = (B // N_CORES) * K          # 512 pairs per core
N_HIDDEN = 6
SUB = 16                            # pairs per DMA subgroup
GRP = 4                             # pairs per PSUM group
UNROLL = 128                        # pairs per outer block

_cache = {}


def _build_bass():
    nc = bacc.Bacc("TRN2")

    ctx_dram = nc.dram_tensor("ctxpairs", [PAIRS, 2, L, D], F32, kind="ExternalInput")
    wt_dram = nc.dram_tensor("wt", [128, N_HIDDEN * 2 * 2 * 128], BF16, kind="ExternalInput")
    wout_dram = nc.dram_tensor("wout", [128, 2], BF16, kind="ExternalInput")
    bias_dram = nc.dram_tensor("bias", [128, N_HIDDEN * 2], F32, kind="ExternalInput")
    bout_dram = nc.dram_tensor("bout", [1, 1], F32, kind="ExternalInput")
    out_dram = nc.dram_tensor("out", [1, PAIRS], F32, kind="ExternalOutput")

    with ExitStack() as ctx:
        tc = ctx.enter_context(tile.TileContext(nc))
        const = ctx.enter_context(tc.tile_pool(name="const", bufs=1))
        raw_pool = ctx.enter_context(tc.tile_pool(name="raw", bufs=3))
        nrm_pool = ctx.enter_context(tc.tile_pool(name="nrm", bufs=2))
        tiny_pool = ctx.enter_context(tc.tile_pool(name="tiny", bufs=4))
        trash_pool = ctx.enter_context(tc.tile_pool(name="trash", bufs=2))
        norm_sb = ctx.enter_context(tc.tile_pool(name="normsb", bufs=2))
        pre_sb = ctx.enter_context(tc.tile_pool(name="presb", bufs=3))
        x_pool = ctx.enter_context(tc.tile_pool(name="xsb", bufs=3))
        y_pool = ctx.enter_context(tc.tile_pool(name="ysb", bufs=3))
        # PSUM: 8 banks total — tp(2) + scr(2) + mlp(4)
        ps_tp = ctx.enter_context(tc.tile_pool(name="pstp", bufs=1, space="PSUM"))
        ps_scr = ctx.enter_context(tc.tile_pool(name="psscr", bufs=2, space="PSUM"))
        ps_mlp = ctx.enter_context(tc.tile_pool(name="psmlp", bufs=2, space="PSUM"))

        wt_sb = const.tile([128, N_HIDDEN, 2, 2, 128], BF16)
        nc.sync.dma_start(out=wt_sb, in_=wt_dram.rearrange("a (i kc mc b) -> a i kc mc b", i=N_HIDDEN, kc=2, mc=2))
        wout_sb = const.tile([128, 2], BF16)
        nc.sync.dma_start(out=wout_sb, in_=wout_dram[:, :])
        bias_sb = const.tile([128, N_HIDDEN * 2], F32)
        nc.sync.dma_start(out=bias_sb, in_=bias_dram[:, :])
        bout_sb = const.tile([1, 1], F32)
        nc.sync.dma_start(out=bout_sb, in_=bout_dram[:, :])
        ident = const.tile([128, 128], F32)
        make_identity(nc, ident)
        identb = const.tile([128, 128], BF16)
        make_identity(nc, identb)
        bout128 = const.tile([1, 1], F32)
        nc.vector.tensor_scalar(out=bout128, in0=bout_sb, scalar1=float(L), scalar2=0.0,
                                op0=AluOpType.mult, op1=AluOpType.add)

        n_sub = UNROLL // SUB

        for g0 in range(0, PAIRS, UNROLL):
            res = nrm_pool.tile([1, UNROLL], F32, tag="res")
            for s in range(n_sub):
                raw = raw_pool.tile([128, SUB, 2, 128], F32, tag="raw")
                nc.sync.dma_start(
                    out=raw,
                    in_=ctx_dram[g0 + s * SUB : g0 + s * SUB + SUB].rearrange("p c l d -> l p c d"),
                )

                # --- norms^2 per (pair, ctx/ent) -> [128, 2*SUB]
                sq = trash_pool.tile([128, SUB, 2, 128], F32, tag="sq")
                nc.gpsimd.tensor_mul(sq, raw, raw)
                nrm2 = nrm_pool.tile([128, 2 * SUB], F32, tag="nrm2")
                nc.vector.tensor_reduce(
                    nrm2.rearrange("a (p c) -> a p c", p=SUB),
                    sq, axis=mybir.AxisListType.X, op=AluOpType.add,
                )
                nc.vector.tensor_scalar(out=nrm2, in0=nrm2, scalar1=1.0 / 128.0,
                                        scalar2=0.0, op0=AluOpType.mult, op1=AluOpType.add)

                # --- rinv = 1/sqrt(nrm2*128) via Newton on x' = nrm2 ~ 1
                yv = tiny_pool.tile([128, 2 * SUB], F32, tag="newty")
                tv = tiny_pool.tile([128, 2 * SUB], F32, tag="newtt")
                nc.vector.tensor_scalar(out=yv, in0=nrm2, scalar1=-0.5, scalar2=1.5,
                                        op0=AluOpType.mult, op1=AluOpType.add)
                for _ in range(3):
                    nc.vector.tensor_mul(tv, yv, yv)
                    nc.vector.tensor_mul(tv, tv, nrm2)
                    nc.vector.tensor_scalar(out=tv, in0=tv, scalar1=-0.5, scalar2=1.5,
                                            op0=AluOpType.mult, op1=AluOpType.add)
                    nc.vector.tensor_mul(yv, yv, tv)
                rinv = tiny_pool.tile([128, 2 * SUB], F32, tag="rinv")
                nc.vector.tensor_scalar(out=rinv, in0=yv, scalar1=float(1.0 / np.sqrt(128.0)),
                                        scalar2=0.0, op0=AluOpType.mult, op1=AluOpType.add)

                # --- normalized rows: one DVE broadcast-mult for all 16 pairs
                normed = norm_sb.tile([128, SUB, 2, 128], F32, tag="normed")
                nc.vector.tensor_tensor(
                    out=normed, in0=raw,
                    in1=rinv.rearrange("a (p c) -> a p c", p=SUB).unsqueeze(3)
                        .broadcast_to([128, SUB, 2, 128]),
                    op=AluOpType.mult,
                )
                entnb = norm_sb.tile([128, SUB, 128], BF16, tag="entnb")
                nc.vector.tensor_copy(entnb, normed[:, :, 1, :])

                x_tiles = []
                for q in range(SUB // GRP):
                    pbase = q * GRP
                    # --- transposes: ctx_nT, ent_nT (fp32, PE)
                    tpc = ps_tp.tile([128, GRP, 128], F32, tag="tpc")
                    tpe = ps_tp.tile([128, GRP, 128], F32, tag="tpe")
                    for j in range(GRP):
                        nc.tensor.transpose(tpc[:, j, :], normed[:, pbase + j, 0, :], ident)
                    for j in range(GRP):
                        nc.tensor.transpose(tpe[:, j, :], normed[:, pbase + j, 1, :], ident)
                    ctxnT = pre_sb.tile([128, GRP, 128], F32, tag="ctxnT")
                    entnT = pre_sb.tile([128, GRP, 128], F32, tag="entnT")
                    nc.vector.tensor_copy(ctxnT, tpc)
                    nc.vector.tensor_copy(entnT, tpe)

                    x_sb = x_pool.tile([128, 2, GRP, 128], BF16, tag="x")
                    # chunk0 bf16 cast on ACT (Copy lives in the tanh table)
                    nc.scalar.activation(
                        out=x_sb[:, 0].rearrange("a g d -> a (g d)"),
                        in_=tpc.rearrange("a g d -> a (g d)"),
                        func=AF.Copy,
                    )

                    # --- similarity (fp32) + argmax one-hot
                    gps = ps_scr.tile([128, GRP, 128], F32, tag="scr")
                    for j in range(GRP):
                        nc.tensor.matmul(gps[:, j, :], lhsT=ctxnT[:, j, :], rhs=entnT[:, j, :])
                    mx = tiny_pool.tile([128, GRP], F32, tag="mx")
                    nc.vector.tensor_reduce(mx, gps, axis=mybir.AxisListType.X, op=AluOpType.max)
                    oh = pre_sb.tile([128, GRP, 128], BF16, tag="oh")
                    nc.vector.tensor_tensor(
                        out=oh, in0=gps,
                        in1=mx.unsqueeze(2).broadcast_to([128, GRP, 128]),
                        op=AluOpType.is_equal,
                    )
                    # --- transpose one-hot (bf16) and gather = ent_n^T @ onehot^T
                    ohT_ps = ps_scr.tile([128, GRP, 128], BF16, tag="scr")
                    for j in range(GRP):
                        nc.tensor.transpose(ohT_ps[:, j, :], oh[:, j, :], identb)
                    ohT = pre_sb.tile([128, GRP, 128], BF16, tag="ohTsb")
                    nc.vector.tensor_copy(ohT, ohT_ps)
                    ch1 = ps_scr.tile([128, GRP, 128], F32, tag="scr")
                    for j in range(GRP):
                        nc.tensor.matmul(ch1[:, j, :], lhsT=entnb[:, pbase + j, :], rhs=ohT[:, j, :])
                    nc.vector.tensor_copy(x_sb[:, 1], ch1)  # chunk1 bf16
                    x_tiles.append(x_sb)

                # --- MLP over 8-pair super-groups: tanh batched [128, 1024]
                for qq in range(SUB // (2 * GRP)):
                    xin = [
                        [x_tiles[2 * qq + g][:, kc].rearrange("a g d -> a (g d)") for kc in range(2)]
                        for g in range(2)
                    ]
                    for i in range(N_HIDDEN):
                        ya = y_pool.tile([128, 2, 2, GRP * 128], BF16, tag="y")
                        for mc in range(2):
                            mm = ps_mlp.tile([128, 2, GRP * 128], F32, tag="mm")
                            for g in range(2):
                                nc.tensor.matmul(mm[:, g, :], lhsT=wt_sb[:, i, 0, mc, :],
                                                 rhs=xin[g][0], start=True, stop=False)
                                nc.tensor.matmul(mm[:, g, :], lhsT=wt_sb[:, i, 1, mc, :],
                                                 rhs=xin[g][1], start=False, stop=True)
                            nc.scalar.activation(
                                out=ya[:, mc].rearrange("a g d -> a (g d)"),
                                in_=mm.rearrange("a g d -> a (g d)"),
                                func=AF.Tanh,
                                bias=bias_sb[:, 2 * i + mc : 2 * i + mc + 1],
                            )
                        xin = [[ya[:, kc, g] for kc in range(2)] for g in range(2)]

                    # --- out = W_out . x7 summed over rows per pair
                    for g in range(2):
                        wo = ps_scr.tile([1, GRP * 128], F32, tag="scr")
                        nc.tensor.matmul(wo, lhsT=wout_sb[:, 0:1], rhs=xin[g][0],
                                         start=True, stop=False)
                        nc.tensor.matmul(wo, lhsT=wout_sb[:, 1:2], rhs=xin[g][1],
                                         start=False, stop=True)
                        col = s * SUB + (2 * qq + g) * GRP
                        nc.vector.tensor_reduce(
                            res[0:1, col : col + GRP],
                            wo.rearrange("a (g d) -> a g d", g=GRP),
                            axis=mybir.AxisListType.X, op=AluOpType.add,
                        )

            # res += L * b_out  (sum over L rows of constant bias)
            nc.vector.tensor_scalar(out=res, in0=res, scalar1=bout128[0:1, 0:1], scalar2=0.0,
                                    op0=AluOpType.add, op1=AluOpType.add)
            nc.sync.dma_start(out=out_dram[0:1, g0 : g0 + UNROLL], in_=res)

    nc.compile()
    return nc


def _prep_weights(Ws, bs, W_out, b_out):
    Ws = np.asarray(Ws, dtype=np.float32)
    bs = np.asarray(bs, dtype=np.float32)
    W_out = np.asarray(W_out, dtype=np.float32)
    b_out = np.asarray(b_out, dtype=np.float32)
    # wt[a, i, kc, mc, b] = Ws[i, mc*128+b, kc*128+a]
    wt = np.transpose(
        Ws.reshape(N_HIDDEN, 2, 128, 2, 128),  # [i, mc, b, kc, a]
        (4, 0, 3, 1, 2),
    ).reshape(128, N_HIDDEN * 2 * 2 * 128)
    wt = np.ascontiguousarray(wt.astype(ml_dtypes.bfloat16))
    wout = np.ascontiguousarray(W_out.reshape(2, 128).T.astype(ml_dtypes.bfloat16))
    bias = np.ascontiguousarray(
        np.transpose(bs.reshape(N_HIDDEN, 2, 128), (2, 0, 1)).reshape(128, N_HIDDEN * 2)
    ).astype(np.float32)
    bout = b_out.reshape(1, 1).astype(np.float32)
    return wt, wout, bias, bout


def make_in_maps(context, Ws, bs, W_out, b_out):
    context = np.ascontiguousarray(np.asarray(context, dtype=np.float32))
    wt, wout, bias, bout = _prep_weights(Ws, bs, W_out, b_out)
    shards = context.reshape(N_CORES, PAIRS, 2, L, D)
    return [
        {"ctxpairs": np.ascontiguousarray(shards[i]), "wt": wt, "wout": wout,
         "bias": bias, "bout": bout}
        for i in range(N_CORES)
    ]


def kernel(context, Ws, bs, W_out, b_out):
    in_maps = make_in_maps(context, Ws, bs, W_out, b_out)
    if "nc" not in _cache:
        _cache["nc"] = _build_bass()
    nc = _cache["nc"]
    r = run_bass_kernel_spmd(nc, in_maps, core_ids=list(range(N_CORES)))
    out = np.concatenate([r.results[i]["out"].reshape(B // N_CORES, K) for i in range(N_CORES)], axis=0)
    return out.astype(np.float32)


if __name__ == "__main__":
    import reference
    inputs = reference.setup_inputs()
    inputs = {k: np.asarray(v) for k, v in inputs.items()}
    expected = np.asarray(reference.reference(**inputs))
    actual = kernel(**inputs)
    err = np.linalg.norm(actual - expected) / np.linalg.norm(expected)
    print("Relative error:", err)


# revision 6
# speedup vs baseline: 2.4753x; 1.3752x over previous
"""Trainium2 Bass kernel for MLP-with-SOM-cosine-similarity (retrieval_knn).

Reference computation per (b, k) pair:
  ctx, ent: [L=128, D=128] slices of context[b, k, 0/1]
  sim[l, m] = cos(ctx[l], ent[m]); idx[l] = argmax_m sim[l, m]
  x = [ctx_n | ent_n[idx]] -> 6x tanh(Linear(256,256)) -> dot W_out -> sum over l
Output: [B=64, K=64] f32.

Strategy: data-parallel over batch dim (8 cores x 8 batches = 512 pairs/core).
Numerics (empirically validated): fp32 similarity (fp16/bf16 flip argmax ->
rel err >1.7e-2), bf16 MLP (fp8 fails at 8.9e-2), Newton rsqrt on DVE (no ACT
table switch).
Engine placement (trace-driven): squares+norm-reduce on GpSimd (tensor_tensor
class ops only - gpsimd tensor_scalar is ~16x slower than modeled), normalize
as one DVE broadcast-mult per 16-pair block, x0 cast DVE, ch1 cast on ACT
(Copy shares the tanh table), tanh on ACT batched [128,1024], final dot =
DVE row-sum of x6 + one tiny W_out matmul per 128-pair block.
Schedule: MLP of block s is emitted AFTER the pre-stage of block s+1
(software pipelining) so the in-order PE/ACT/DVE queues never drain at block
boundaries.
"""

from contextlib import ExitStack

import numpy as np
import ml_dtypes

import concourse.bass as bass
import concourse.bacc as bacc
import concourse.tile as tile
from concourse import mybir
from concourse.alu_op_type import AluOpType
from concourse.bass_utils import run_bass_kernel_spmd
from concourse.masks import make_identity

BF16 = mybir.dt.bfloat16
F32 = mybir.dt.float32
AF = mybir.ActivationFunctionType

B, K, L, D = 64, 64, 128, 128
N_CORES = 8
PAIRS = (B // N_CORES) * K          # 512 pairs per core
N_HIDDEN = 6
SUB = 16                            # pairs per DMA subgroup
GRP = 4                             # pairs per PSUM group
UNROLL = 128                        # pairs per output block

_cache = {}


def _build_bass():
    nc = bacc.Bacc("TRN2")

    ctx_dram = nc.dram_tensor("ctxpairs", [PAIRS, 2, L, D], F32, kind="ExternalInput")
    wt_dram = nc.dram_tensor("wt", [128, N_HIDDEN * 2 * 2 * 128], BF16, kind="ExternalInput")
    wout_dram = nc.dram_tensor("wout", [128, 2], BF16, kind="ExternalInput")
    bias_dram = nc.dram_tensor("bias", [128, N_HIDDEN * 2], F32, kind="ExternalInput")
    bout_dram = nc.dram_tensor("bout", [1, 1], F32, kind="ExternalInput")
    out_dram = nc.dram_tensor("out", [1, PAIRS], F32, kind="ExternalOutput")

    with ExitStack() as ctx:
        tc = ctx.enter_context(tile.TileContext(nc))
        const = ctx.enter_context(tc.tile_pool(name="const", bufs=1))
        raw_pool = ctx.enter_context(tc.tile_pool(name="raw", bufs=3))
        nrm_pool = ctx.enter_context(tc.tile_pool(name="nrm", bufs=2))
        tiny_pool = ctx.enter_context(tc.tile_pool(name="tiny", bufs=4))
        trash_pool = ctx.enter_context(tc.tile_pool(name="trash", bufs=2))
        norm_sb = ctx.enter_context(tc.tile_pool(name="normsb", bufs=2))
        pre_sb = ctx.enter_context(tc.tile_pool(name="presb", bufs=3))
        x_pool = ctx.enter_context(tc.tile_pool(name="xsb", bufs=10))
        y_pool = ctx.enter_context(tc.tile_pool(name="ysb", bufs=3))
        s_pool = ctx.enter_context(tc.tile_pool(name="ssb", bufs=2))
        # PSUM: 8 banks total — tp(2) + scr(2) + mlp(4)
        ps_tp = ctx.enter_context(tc.tile_pool(name="pstp", bufs=1, space="PSUM"))
        ps_scr = ctx.enter_context(tc.tile_pool(name="psscr", bufs=2, space="PSUM"))
        ps_mlp = ctx.enter_context(tc.tile_pool(name="psmlp", bufs=2, space="PSUM"))

        wt_sb = const.tile([128, N_HIDDEN, 2, 2, 128], BF16)
        nc.sync.dma_start(out=wt_sb, in_=wt_dram.rearrange("a (i kc mc b) -> a i kc mc b", i=N_HIDDEN, kc=2, mc=2))
        wout_sb = const.tile([128, 2], BF16)
        nc.sync.dma_start(out=wout_sb, in_=wout_dram[:, :])
        bias_sb = const.tile([128, N_HIDDEN * 2], F32)
        nc.sync.dma_start(out=bias_sb, in_=bias_dram[:, :])
        bout_sb = const.tile([1, 1], F32)
        nc.sync.dma_start(out=bout_sb, in_=bout_dram[:, :])
        ident = const.tile([128, 128], F32)
        make_identity(nc, ident)
        identb = const.tile([128, 128], BF16)
        make_identity(nc, identb)
        bout128 = const.tile([1, 1], F32)
        nc.vector.tensor_scalar(out=bout128, in0=bout_sb, scalar1=float(L), scalar2=0.0,
                                op0=AluOpType.mult, op1=AluOpType.add)

        n_sub = PAIRS // SUB                      # 32
        subs_per_block = UNROLL // SUB            # 8
        state = {}                                # per-block res/s tiles

        def pre_stage(s):
            """DMA + norms + normalize + per-group sim/argmax/gather for SUB s.
            Returns x_tiles (list of 4 x_sb tiles, 4 pairs each)."""
            raw = raw_pool.tile([128, SUB, 2, 128], F32, tag="raw")
            nc.sync.dma_start(
                out=raw,
                in_=ctx_dram[s * SUB : (s + 1) * SUB].rearrange("p c l d -> l p c d"),
            )

            # --- norms^2 per (pair, ctx/ent) -> [128, 2*SUB]  (GpSimd)
            sq = trash_pool.tile([128, SUB, 2, 128], F32, tag="sq")
            nc.gpsimd.tensor_mul(sq, raw, raw)
            nrm2 = nrm_pool.tile([128, 2 * SUB], F32, tag="nrm2")
            nc.vector.tensor_reduce(
                nrm2.rearrange("a (p c) -> a p c", p=SUB),
                sq, axis=mybir.AxisListType.X, op=AluOpType.add,
            )

            # --- rinv = 1/sqrt(nrm2) via Newton, y0 from nrm2/128 ~ 1 (DVE)
            yv = tiny_pool.tile([128, 2 * SUB], F32, tag="newty")
            tv = tiny_pool.tile([128, 2 * SUB], F32, tag="newtt")
            s128 = float(1.0 / np.sqrt(128.0))
            nc.vector.tensor_scalar(out=yv, in0=nrm2, scalar1=-0.5 / 128.0 * s128,
                                    scalar2=1.5 * s128,
                                    op0=AluOpType.mult, op1=AluOpType.add)
            for _ in range(3):
                nc.vector.tensor_mul(tv, yv, yv)
                nc.vector.tensor_mul(tv, tv, nrm2)
                nc.vector.tensor_scalar(out=tv, in0=tv, scalar1=-0.5, scalar2=1.5,
                                        op0=AluOpType.mult, op1=AluOpType.add)
                nc.vector.tensor_mul(yv, yv, tv)

            # --- normalized rows: one DVE broadcast-mult for all 16 pairs
            normed = norm_sb.tile([128, SUB, 2, 128], F32, tag="normed")
            nc.vector.tensor_tensor(
                out=normed, in0=raw,
                in1=yv.rearrange("a (p c) -> a p c", p=SUB).unsqueeze(3)
                    .broadcast_to([128, SUB, 2, 128]),
                op=AluOpType.mult,
            )
            entnb = norm_sb.tile([128, SUB, 128], BF16, tag="entnb")
            nc.vector.tensor_copy(entnb, normed[:, :, 1, :])

            x_tiles = []
            for q in range(SUB // GRP):
                pbase = q * GRP
                # --- transposes: ctx_nT, ent_nT (fp32, PE)
                tpc = ps_tp.tile([128, GRP, 128], F32, tag="tpc")
                tpe = ps_tp.tile([128, GRP, 128], F32, tag="tpe")
                for j in range(GRP):
                    nc.tensor.transpose(tpc[:, j, :], normed[:, pbase + j, 0, :], ident)
                for j in range(GRP):
                    nc.tensor.transpose(tpe[:, j, :], normed[:, pbase + j, 1, :], ident)
                ctxnT = pre_sb.tile([128, GRP, 128], F32, tag="ctxnT")
                entnT = pre_sb.tile([128, GRP, 128], F32, tag="entnT")
                nc.vector.tensor_copy(ctxnT, tpc)
                nc.vector.tensor_copy(entnT, tpe)

                x_sb = x_pool.tile([128, 2, GRP, 128], BF16, tag="x")
                nc.vector.tensor_copy(x_sb[:, 0], tpc)  # chunk0 bf16 (DVE)

                # --- similarity (fp32) + argmax one-hot
                gps = ps_scr.tile([128, GRP, 128], F32, tag="scr")
                for j in range(GRP):
                    nc.tensor.matmul(gps[:, j, :], lhsT=ctxnT[:, j, :], rhs=entnT[:, j, :])
                mx = tiny_pool.tile([128, GRP], F32, tag="mx")
                nc.vector.tensor_reduce(mx, gps, axis=mybir.AxisListType.X, op=AluOpType.max)
                oh = pre_sb.tile([128, GRP, 128], BF16, tag="oh")
                nc.vector.tensor_tensor(
                    out=oh, in0=gps,
                    in1=mx.unsqueeze(2).broadcast_to([128, GRP, 128]),
                    op=AluOpType.is_equal,
                )
                # --- transpose one-hot (bf16) and gather = ent_n^T @ onehot^T
                ohT_ps = ps_scr.tile([128, GRP, 128], BF16, tag="scr")
                for j in range(GRP):
                    nc.tensor.transpose(ohT_ps[:, j, :], oh[:, j, :], identb)
                ohT = pre_sb.tile([128, GRP, 128], BF16, tag="ohTsb")
                nc.vector.tensor_copy(ohT, ohT_ps)
                ch1 = ps_scr.tile([128, GRP, 128], F32, tag="scr")
                for j in range(GRP):
                    nc.tensor.matmul(ch1[:, j, :], lhsT=entnb[:, pbase + j, :], rhs=ohT[:, j, :])
                nc.scalar.activation(
                    out=x_sb[:, 1].rearrange("a g d -> a (g d)"),
                    in_=ch1.rearrange("a g d -> a (g d)"),
                    func=AF.Copy,
                )  # chunk1 bf16 (ACT)
                x_tiles.append(x_sb)
            return x_tiles

        def mlp_stage(s, x_tiles):
            """6-layer MLP + x6 row-sums for SUB s; block epilogue on last SUB."""
            blk = s // subs_per_block
            if s % subs_per_block == 0:
                state["s_tile"] = s_pool.tile([128, 2, UNROLL], BF16, tag="stile", name="s_tile")
            s_tile = state["s_tile"]

            for qq in range(SUB // (2 * GRP)):
                xin = [
                    [x_tiles[2 * qq + g][:, kc].rearrange("a g d -> a (g d)") for kc in range(2)]
                    for g in range(2)
                ]
                for i in range(N_HIDDEN):
                    ya = y_pool.tile([128, 2, 2, GRP * 128], BF16, tag="y")
                    for mc in range(2):
                        mm = ps_mlp.tile([128, 2, GRP * 128], F32, tag="mm")
                        for g in range(2):
                            nc.tensor.matmul(mm[:, g, :], lhsT=wt_sb[:, i, 0, mc, :],
                                             rhs=xin[g][0], start=True, stop=False)
                            nc.tensor.matmul(mm[:, g, :], lhsT=wt_sb[:, i, 1, mc, :],
                                             rhs=xin[g][1], start=False, stop=True)
                        nc.scalar.activation(
                            out=ya[:, mc].rearrange("a g d -> a (g d)"),
                            in_=mm.rearrange("a g d -> a (g d)"),
                            func=AF.Tanh,
                            bias=bias_sb[:, 2 * i + mc : 2 * i + mc + 1],
                        )
                    xin = [[ya[:, kc, g] for kc in range(2)] for g in range(2)]

                # --- per-pair row-sums of x6 (DVE); s_tile[:, kc, pair] = sum_l x6
                col = (s % subs_per_block) * SUB + qq * 2 * GRP
                with nc.allow_low_precision(reason="x6 row-sums: fp32 internal accum, bf16 store feeds a bf16 dot"):
                    nc.vector.tensor_reduce(
                        s_tile[:, :, col : col + 2 * GRP].rearrange("a k (g p) -> a k g p", g=2),
                        ya.rearrange("a k g (p l) -> a k g p l", p=GRP),
                        axis=mybir.AxisListType.X, op=AluOpType.add,
                    )

            if s % subs_per_block == subs_per_block - 1:
                # --- out[pair] = W_out . s[:, pair] + L*b_out
                out_ps = ps_scr.tile([1, UNROLL], F32, tag="scr")
                nc.tensor.matmul(out_ps, lhsT=wout_sb[:, 0:1], rhs=s_tile[:, 0, :],
                                 start=True, stop=False)
                nc.tensor.matmul(out_ps, lhsT=wout_sb[:, 1:2], rhs=s_tile[:, 1, :],
                                 start=False, stop=True)
                res = nrm_pool.tile([1, UNROLL], F32, tag="res")
                nc.vector.tensor_scalar(out=res, in0=out_ps, scalar1=bout128[0:1, 0:1],
                                        scalar2=0.0, op0=AluOpType.add, op1=AluOpType.add)
                nc.sync.dma_start(out=out_dram[0:1, blk * UNROLL : (blk + 1) * UNROLL], in_=res)

        # software pipeline: pre(s+1) is emitted before mlp(s)
        pending = None
        for s in range(n_sub):
            x_tiles = pre_stage(s)
            if pending is not None:
                mlp_stage(*pending)
            pending = (s, x_tiles)
        mlp_stage(*pending)

    nc.compile()
    return nc


def _prep_weights(Ws, bs, W_out, b_out):
    Ws = np.asarray(Ws, dtype=np.float32)
    bs = np.asarray(bs, dtype=np.float32)
    W_out = np.asarray(W_out, dtype=np.float32)
    b_out = np.asarray(b_out, dtype=np.float32)
    # wt[a, i, kc, mc, b] = Ws[i, mc*128+b, kc*128+a]
    wt = np.transpose(
        Ws.reshape(N_HIDDEN, 2, 128, 2, 128),  # [i, mc, b, kc, a]
        (4, 0, 3, 1, 2),
    ).reshape(128, N_HIDDEN * 2 * 2 * 128)
    wt = np.ascontiguousarray(wt.astype(ml_dtypes.bfloat16))
    wout = np.ascontiguousarray(W_out.reshape(2, 128).T.astype(ml_dtypes.bfloat16))
    bias = np.ascontiguousarray(
        np.transpose(bs.reshape(N_HIDDEN, 2, 128), (2, 0, 1)).reshape(128, N_HIDDEN * 2)
    ).astype(np.float32)
    bout = b_out.reshape(1, 1).astype(np.float32)
    return wt, wout, bias, bout


def make_in_maps(context, Ws, bs, W_out, b_out):
    context = np.ascontiguousarray(np.asarray(context, dtype=np.float32))
    wt, wout, bias, bout = _prep_weights(Ws, bs, W_out, b_out)
    shards = context.reshape(N_CORES, PAIRS, 2, L, D)
    return [
        {"ctxpairs": np.ascontiguousarray(shards[i]), "wt": wt, "wout": wout,
         "bias": bias, "bout": bout}
        for i in range(N_CORES)
    ]


def kernel(context, Ws, bs, W_out, b_out):
    in_maps = make_in_maps(context, Ws, bs, W_out, b_out)
    if "nc" not in _cache:
        _cache["nc"] = _build_bass()
    nc = _cache["nc"]
    r = run_bass_kernel_spmd(nc, in_maps, core_ids=list(range(N_CORES)))
    out = np.concatenate([r.results[i]["out"].reshape(B // N_CORES, K) for i in range(N_CORES)], axis=0)
    return out.astype(np.float32)


if __name__ == "__main__":
    import reference
    inputs = reference.setup_inputs()
    inputs = {k: np.asarray(v) for k, v in inputs.items()}
    expected = np.asarray(reference.reference(**inputs))
    actual = kernel(**inputs)
    err = np.linalg.norm(actual - expected) / np.linalg.norm(expected)
    print("Relative error:", err)
